# revision 1
# baseline (speedup 1.0000x reference)
"""Trainium2 Bass kernel for a pre-norm transformer encoder layer with GEGLU FFN.

Sharding: token-data-parallel over 8 cores. Core c handles batch c//4 and the
512-token slice (c%4) of that batch. K/V for the full 2048-token sequence are
exchanged with one AllGather per 4-core group. All activations are kept
feature-major [feature(partition), token(free)] so the matmul chain composes
with zero on-chip transposes; softmax runs on transposed scores with the
denominator computed by a ones-column in the AV matmul.
"""

import numpy as np

import concourse.bass as bass
import concourse.mybir as mybir
import concourse.tile as tile
from concourse import bacc
from concourse.bass_utils import run_bass_kernel_spmd

P = 128
D = 64  # head dim (fixed)
F32 = mybir.dt.float32
F32R = mybir.dt.float32r
BF16 = mybir.dt.bfloat16
AF = mybir.ActivationFunctionType
ALU = mybir.AluOpType

# full-size problem config
FULL = dict(E=1024, T_OWN=512, FF=4096, R=4)
EPS = 1e-5
N_CORES = 8
GROUPS = [[0, 1, 2, 3], [4, 5, 6, 7]]


def build(nc, E, T_OWN, FF, R):
    H = E // D            # heads
    n_et = E // P         # e-tiles == head-pairs == out-tiles
    n_ff = FF // P        # ff tiles per half (a / gate)
    T = R * T_OWN         # full sequence
    n_tt = T // P         # t2 tiles
    n_to = T_OWN // P     # own-token tiles
    T1 = T_OWN            # free dim of most matmuls (<= 512)
    assert T1 <= 512
    OC_W = min(512, E)
    n_oc = E // OC_W

    # ---- DRAM I/O ----
    # srcT holds the full batch sequence feature-major, with the core's own
    # 512-token chunk rotated to column-block 0 (so one SPMD program serves
    # all cores; softmax ordering over t2 is permutation-invariant).
    OWN_R = 0
    srcT = nc.dram_tensor("srcT", [E, T], F32R, kind="ExternalInput")
    wq = nc.dram_tensor("wq", [n_et, P, n_et, P], BF16, kind="ExternalInput")
    wk = nc.dram_tensor("wk", [n_et, P, n_et, P], BF16, kind="ExternalInput")
    wv = nc.dram_tensor("wv", [n_oc, n_et, P, OC_W], BF16, kind="ExternalInput")
    wo = nc.dram_tensor("wo", [n_et, P, n_et, P], F32R, kind="ExternalInput")
    w1 = nc.dram_tensor("w1", [2 * n_ff, P, n_et, P], F32R, kind="ExternalInput")
    w2 = nc.dram_tensor("w2", [n_et, P, n_ff, P], BF16, kind="ExternalInput")
    b1d = nc.dram_tensor("b1d", [P, 2 * n_ff], F32, kind="ExternalInput")
    b2d = nc.dram_tensor("b2d", [P, n_et], F32, kind="ExternalInput")
    lnv = nc.dram_tensor("lnv", [P, 4 * n_et], F32, kind="ExternalInput")
    outT = nc.dram_tensor("outT", [E, T_OWN], F32, kind="ExternalOutput")

    def mm(ps, lhsT, rhs, start, stop):
        nc.tensor.matmul(ps, lhsT, rhs, start=start, stop=stop)

    with nc.allow_low_precision(reason="f32r/bf16 tiles feeding PE; fp32 PSUM accumulation"), \
            tile.TileContext(nc) as tc, tc.tile_pool(name="consts", bufs=1) as constp:
        def single(shape, name, dt=F32):
            return constp.tile(shape, dt, name=name, tag=name)

        # ---- constants / small params ----
        ones_col = single([P, 1], "ones_col", F32R)
        nc.vector.memset(ones_col[:].bitcast(F32), 1.0)
        ones_row = single([1, P], "ones_row", F32R)
        nc.vector.memset(ones_row[:].bitcast(F32), 1.0)
        eps_t = single([1, 1], "eps_t")
        nc.vector.memset(eps_t[:], EPS)

        lnc = single([P, 4 * n_et], "lnc")
        nc.sync.dma_start(lnc[:], lnv[:])
        b1c = single([P, 2 * n_ff], "b1c")
        nc.sync.dma_start(b1c[:], b1d[:])
        b2c = single([P, n_et], "b2c")
        nc.sync.dma_start(b2c[:], b2d[:])

        from contextlib import ExitStack
        with ExitStack() as es:
            pool = lambda **kw: es.enter_context(tc.tile_pool(**kw))
            srcp = pool(name="srcp", bufs=9)
            hTp = pool(name="hT", bufs=10)              # h1 chunks / attnT / h2
            qp = pool(name="qp", bufs=n_et)             # qT bf16
            bigp = pool(name="big", bufs=n_ff)          # fT
            xp = pool(name="xp", bufs=n_et)
            wblkp = pool(name="wblk", bufs=4)
            wkresp = pool(name="wkres", bufs=n_et)
            wvresp = pool(name="wvres", bufs=n_oc * n_et)
            w2p = pool(name="w2p", bufs=4)
            kslabp = pool(name="kslab", bufs=2)
            vsbp = pool(name="vsb", bufs=4)
            probsp = pool(name="probs", bufs=8)
            evp = pool(name="ev", bufs=5)
            rbp = pool(name="rb", bufs=2)
            statsp = pool(name="stats", bufs=6)
            psS = pool(name="psS", bufs=4, space="PSUM")
            psA = pool(name="psA", bufs=2, space="PSUM")
            psM = pool(name="psM", bufs=2, space="PSUM")
            dram = pool(name="dram", bufs=1, space="DRAM")

            _gp = [0]
            _gp_targets = [(psS, "psS"), (psM, "psM"), (psS, "psS"), (psA, "psA")]

            def group_psum(name, width=T1):
                # big matmul-group accumulators rotate across all three pools
                # (weighted by bufs) so every bank stays in play outside the
                # attention loop
                pool_, tag = _gp_targets[_gp[0] % len(_gp_targets)]
                _gp[0] += 1
                return pool_.tile([P, width], F32, name=name, tag=tag)

            # local DRAM scratch for full-sequence K (feature-major) and V
            # (head-major); per-hp / per-head tiles so attention loads only
            # wait on their own producers
            kT_dram = [dram.tile([P, T], BF16, name=f"kT_dram{i}") for i in range(n_et)]
            v_dram = [dram.tile([T, D], BF16, name=f"v_dram{i}") for i in range(H)]

            def layer_norm(x_tiles, wcol, bcol, label, out_pool, width, out_dt=F32R):
                """x_tiles: n_et SBUF [P, width] feature-major chunk."""
                m_ps = psS.tile([1, width], F32, name=f"mps_{label}", tag="psS")
                s_ps = psS.tile([1, width], F32, name=f"sps_{label}", tag="psS")
                for kt in range(n_et):
                    mm(m_ps[:], ones_col[:], x_tiles[kt][:], kt == 0, kt == n_et - 1)
                for kt in range(n_et):
                    sq = evp.tile([P, width], F32R, name=f"sq_{label}{kt}", tag="ev")
                    nc.scalar.square(sq[:], x_tiles[kt][:])
                    mm(s_ps[:], ones_col[:], sq[:], kt == 0, kt == n_et - 1)
                m_sb = statsp.tile([1, width], F32R, name=f"m_{label}", tag="st")
                nc.vector.tensor_scalar_mul(m_sb[:], m_ps[:], 1.0 / E)
                msq = statsp.tile([1, width], F32, name=f"msq_{label}", tag="st")
                nc.vector.tensor_mul(msq[:], m_sb[:], m_sb[:])
                var = statsp.tile([1, width], F32, name=f"var_{label}", tag="st")
                nc.vector.scalar_tensor_tensor(
                    var[:], s_ps[:], 1.0 / E, msq[:], ALU.mult, ALU.subtract)
                sd = statsp.tile([1, width], F32, name=f"sd_{label}", tag="st")
                nc.scalar.activation(sd[:], var[:], AF.Sqrt, bias=eps_t[0:1, 0:1])
                rinv = statsp.tile([1, width], F32R, name=f"ri_{label}", tag="st")
                nc.vector.reciprocal(rinv[:], sd[:])
                mr_sb = statsp.tile([1, width], F32R, name=f"mr_{label}", tag="st")
                nc.vector.tensor_mul(mr_sb[:], m_sb[:], rinv[:])
                # broadcast rstd and mean*rstd across partitions via K=1 matmuls
                rs_ps = psS.tile([P, width], F32, name=f"rb_{label}", tag="psS")
                mm(rs_ps[:], ones_row[0:1, :], rinv[:], True, True)
                mr_ps = psS.tile([P, width], F32, name=f"mb_{label}", tag="psS")
                mm(mr_ps[:], ones_row[0:1, :], mr_sb[:], True, True)
                rs_sb = evp.tile([P, width], F32, name=f"rsb_{label}", tag="ev")
                nc.vector.tensor_copy(rs_sb[:], rs_ps[:])
                mrb_sb = evp.tile([P, width], F32, name=f"mrb_{label}", tag="ev")
                nc.vector.tensor_copy(mrb_sb[:], mr_ps[:])
                h_tiles = []
                for kt in range(n_et):
                    # h = (x*rstd - m*rstd)*g + b; g,b per-partition via ACT
                    t = evp.tile([P, width], F32, name=f"xc_{label}{kt}", tag="ev")
                    nc.vector.tensor_mul(t[:], x_tiles[kt][:], rs_sb[:])
                    t2 = evp.tile([P, width], F32, name=f"xs_{label}{kt}", tag="ev")
                    nc.vector.tensor_sub(t2[:], t[:], mrb_sb[:])
                    h = out_pool.tile([P, width], out_dt, name=f"h_{label}{kt}", tag="hT")
                    nc.scalar.activation(h[:], t2[:], AF.Identity,
                                         bias=bcol[:, kt:kt + 1],
                                         scale=wcol[:, kt:kt + 1])
                    h_tiles.append(h)
                return h_tiles

            # ---- prologue: chunk-0 src tiles first so LN1 starts immediately ----
            src_c0 = []
            for kt in range(n_et):
                sx = srcp.tile([P, T1], F32R, name=f"src0_{kt}", tag="src")
                nc.sync.dma_start(sx[:], srcT[kt * P:(kt + 1) * P, 0:T1])
                src_c0.append(sx)

            # ---- resident bf16 K/V weights (loaded once) ----
            wk_res = []
            for ot in range(n_et):
                wkr = wkresp.tile([P, n_et, P], BF16, name=f"wkr{ot}", tag="wkr")
                nc.sync.dma_start(wkr[:], wk[ot])
                wk_res.append(wkr)
            wv_res = {}
            for oc in range(n_oc):
                for kt in range(n_et):
                    wvr = wvresp.tile([P, OC_W], BF16, name=f"wvr{oc}_{kt}", tag="wvr")
                    nc.sync.dma_start(wvr[:], wv[oc, kt])
                    wv_res[(oc, kt)] = wvr

            # ---- LN1 + K/V over full sequence, chunked; Q for own tokens ----
            # own chunk index is baked into the data: srcT column-block OWN_R
            q_sb = [None] * n_et
            for r in range(R):
              with nc.named_scope(f"lnkv{r}"):
                  if r == 0:
                      xs = src_c0
                  else:
                      xs = []
                      for kt in range(n_et):
                          sx = srcp.tile([P, T1], F32R, name=f"src{r}_{kt}", tag="src")
                          nc.sync.dma_start(sx[:], srcT[kt * P:(kt + 1) * P,
                                                        r * T1:(r + 1) * T1])
                          xs.append(sx)
                  h1r = layer_norm(xs, lnc[:, 0:n_et], lnc[:, n_et:2 * n_et],
                                   f"l1c{r}", hTp, T1, out_dt=BF16)
                  # Q only for the OWN chunk (srcT own block marked by prep: always block OWN_R)
                  if r == OWN_R:
                      for ot in range(n_et):
                          wq_sb = wblkp.tile([P, n_et, P], BF16, name=f"wq{ot}", tag="wblk")
                          nc.sync.dma_start(wq_sb[:], wq[ot])
                          ps = group_psum(f"psq{ot}")
                          for kt in range(n_et):
                              mm(ps[:], wq_sb[:, kt, :], h1r[kt][:], kt == 0, kt == n_et - 1)
                          q = qp.tile([P, T1], BF16, name=f"q{ot}", tag="q")
                          nc.vector.tensor_copy(q[:], ps[:])
                          q_sb[ot] = q
                  # V chunk: v[heads, r-block, :]
                  for oc in range(n_oc):
                      for to in range(n_to):
                          ps = group_psum(f"psv{r}_{oc}_{to}", OC_W)
                          for kt in range(n_et):
                              mm(ps[:], h1r[kt][:, to * P:(to + 1) * P], wv_res[(oc, kt)][:],
                                 kt == 0, kt == n_et - 1)
                          vev = evp.tile([P, OC_W], BF16, name=f"vev{r}_{oc}_{to}", tag="evb")
                          nc.vector.tensor_copy(vev[:], ps[:])
                          for hh in range(OC_W // D):
                              h_idx = oc * (OC_W // D) + hh
                              nc.sync.dma_start(
                                  v_dram[h_idx][r * T1 + to * P: r * T1 + (to + 1) * P, :],
                                  vev[:, hh * D:(hh + 1) * D])
                  # K chunk: kT[:, r-block]
                  for ot in range(n_et):
                      ps = group_psum(f"psk{r}_{ot}")
                      for kt in range(n_et):
                          mm(ps[:], wk_res[ot][:, kt, :], h1r[kt][:], kt == 0, kt == n_et - 1)
                      kev = evp.tile([P, T1], BF16, name=f"kev{r}_{ot}", tag="evb")
                      nc.scalar.copy(kev[:], ps[:])
                      nc.sync.dma_start(
                          kT_dram[ot][:, r * T1:(r + 1) * T1], kev[:])

            # ---- attention, one head-pair (=128 feature rows) at a time ----
            with nc.named_scope("attn"):
                attn_sb = []

                def emit_normalize(a, att_ps, hp):
                    recs, rbpss, rbs = [], [], []
                    for hl in range(2):
                        rec = statsp.tile([1, T1], F32R, name=f"rec{hp}_{hl}", tag="st")
                        nc.vector.reciprocal(rec[:], att_ps[hl][D:D + 1, :])
                        recs.append(rec)
                    for hl in range(2):
                        rbps = psM.tile([P, T1], F32, name=f"rbp{hp}_{hl}", tag="psM")
                        mm(rbps[0:D, :], ones_row[0:1, 0:D], recs[hl][:], True, True)
                        rbpss.append(rbps)
                    for hl in range(2):
                        rb_sb = rbp.tile([D, T1], F32, name=f"rbs{hp}_{hl}", tag="rb")
                        nc.scalar.copy(rb_sb[:], rbpss[hl][0:D, :])
                        rbs.append(rb_sb)
                    for hl in range(2):
                        nc.vector.tensor_mul(a[hl * D:(hl + 1) * D, :],
                                             att_ps[hl][0:D, :], rbs[hl][:])

                pending = None
                for hp in range(n_et):
                    kslab = kslabp.tile([P, T], BF16, name=f"ks{hp}", tag="ks")
                    nc.sync.dma_start(kslab[:], kT_dram[hp][:])
                    vsb = []
                    for hl in range(2):
                        h_idx = hp * 2 + hl
                        v = vsbp.tile([P, n_tt, D + 1], BF16, name=f"v{hp}_{hl}", tag="vs")
                        nc.sync.dma_start(
                            v[:, :, 0:D],
                            v_dram[h_idx][:].rearrange("(tt p) d -> p tt d", p=P))
                        nc.gpsimd.memset(v[:, :, D:D + 1], 1.0)
                        vsb.append(v)
                    att_ps = [psA.tile([D + 1, T1], F32, name=f"pa{hp}_{hl}", tag="psA")
                              for hl in range(2)]
                    for tt in range(n_tt):
                        for hl in range(2):
                            sc = psS.tile([P, T1], F32, name=f"sc{hp}_{tt}_{hl}", tag="psS")
                            mm(sc[:],
                               kslab[hl * D:(hl + 1) * D, tt * P:(tt + 1) * P],
                               q_sb[hp][hl * D:(hl + 1) * D, :], True, True)
                            pr = probsp.tile([P, T1], BF16, name=f"pr{hp}_{tt}_{hl}", tag="pr")
                            nc.scalar.activation(pr[:], sc[:], AF.Exp, scale=0.125)
                            if tt == min(4, n_tt // 2) and hl == 0 and pending is not None:
                                emit_normalize(*pending)
                                pending = None
                            mm(att_ps[hl][:], vsb[hl][:, tt, :], pr[:],
                               tt == 0, tt == n_tt - 1)
                    a = hTp.tile([P, T1], F32R, name=f"attn{hp}", tag="hT")
                    attn_sb.append(a)
                    pending = (a, att_ps, hp)
                emit_normalize(*pending)

            # ---- Wo + residual -> xT ----
            with nc.named_scope("wo"):
                x_sb = []
                for ot in range(n_et):
                    wo_sb = wblkp.tile([P, n_et, P], F32R, name=f"wo{ot}", tag="wblk")
                    nc.sync.dma_start(wo_sb[:], wo[ot])
                    ps = group_psum(f"pso{ot}")
                    for kt in range(n_et):
                        mm(ps[:], wo_sb[:, kt, :], attn_sb[kt][:], kt == 0, kt == n_et - 1)
                    so = evp.tile([P, T1], F32R, name=f"so{ot}", tag="ev")
                    nc.sync.dma_start(so[:], srcT[ot * P:(ot + 1) * P,
                                                  OWN_R * T1:(OWN_R + 1) * T1])
                    x = xp.tile([P, T1], F32R, name=f"x{ot}", tag="x")
                    nc.vector.tensor_add(x[:], ps[:], so[:])
                    x_sb.append(x)

            # ---- LN2 ----
            with nc.named_scope("ln2"):
                h2 = layer_norm(x_sb, lnc[:, 2 * n_et:3 * n_et],
                                lnc[:, 3 * n_et:4 * n_et], "l2", hTp, T1)

            # ---- FFN: u = h2 @ W1.T + b1 (transposed), GEGLU ----
            with nc.named_scope("ffn1"):
                f_sb = []
                for pt in range(n_ff):
                    w1a = wblkp.tile([P, n_et, P], F32R, name=f"w1a{pt}", tag="wblk")
                    nc.sync.dma_start(w1a[:], w1[pt])
                    w1g = wblkp.tile([P, n_et, P], F32R, name=f"w1g{pt}", tag="wblk")
                    nc.sync.dma_start(w1g[:], w1[n_ff + pt])
                    psa = group_psum(f"psa{pt}")
                    for kt in range(n_et):
                        mm(psa[:], w1a[:, kt, :], h2[kt][:], kt == 0, kt == n_et - 1)
                    psg = group_psum(f"psg{pt}")
                    for kt in range(n_et):
                        mm(psg[:], w1g[:, kt, :], h2[kt][:], kt == 0, kt == n_et - 1)
                    gel = evp.tile([P, T1], F32, name=f"gel{pt}", tag="ev")
                    nc.scalar.activation(gel[:], psg[:], AF.Gelu,
                                         bias=b1c[:, n_ff + pt:n_ff + pt + 1])
                    f = bigp.tile([P, T1], BF16, name=f"f{pt}", tag="big")
                    nc.vector.scalar_tensor_tensor(
                        f[:], psa[:], b1c[:, pt:pt + 1], gel[:], ALU.add, ALU.mult)
                    f_sb.append(f)

            # ---- W2 + b2 + residual -> outT ----
            with nc.named_scope("w2out"):
                n_ffh = max(1, n_ff // 4)
                for ot in range(n_et):
                    ps = group_psum(f"psy{ot}")
                    w2h = []
                    for half in range(n_ff // n_ffh):
                        w = w2p.tile([P, n_ffh, P], BF16, name=f"w2_{ot}_{half}", tag="w2")
                        nc.sync.dma_start(
                            w[:], w2[ot, :, half * n_ffh:(half + 1) * n_ffh, :])
                        w2h.append(w)
                    for c in range(n_ff):
                        mm(ps[:], w2h[c // n_ffh][:, c % n_ffh, :], f_sb[c][:],
                           c == 0, c == n_ff - 1)
                    y = evp.tile([P, T1], F32, name=f"y{ot}", tag="ev")
                    nc.vector.scalar_tensor_tensor(
                        y[:], ps[:], b2c[:, ot:ot + 1], x_sb[ot][:], ALU.add, ALU.add)
                    nc.sync.dma_start(outT[ot * P:(ot + 1) * P, :], y[:])

    return nc


def prep_inputs(src, Wq, Wk, Wv, Wo, W1, b1, W2, b2,
                ln1_w, ln1_b, ln2_w, ln2_b, E, T_OWN, FF, R):
    """Host-side: transpose/retile weights, shard src. Returns per-core in_maps."""
    n_et = E // P
    n_ff = FF // P
    OC_W = min(512, E)
    n_oc = E // OC_W
    import ml_dtypes
    bf16 = ml_dtypes.bfloat16
    c = np.ascontiguousarray
    shared = {
        "wq": c(Wq.reshape(n_et, P, n_et, P).transpose(0, 3, 2, 1)).astype(bf16),
        "wk": c(Wk.reshape(n_et, P, n_et, P).transpose(0, 3, 2, 1)).astype(bf16),
        "wv": c(Wv.reshape(n_oc, OC_W, n_et, P).transpose(0, 2, 3, 1)).astype(bf16),
        "wo": c(Wo.reshape(n_et, P, n_et, P).transpose(0, 3, 2, 1)),
        "w1": c(W1.reshape(2 * n_ff, P, n_et, P).transpose(0, 3, 2, 1)),
        "w2": c(W2.reshape(n_et, P, n_ff, P).transpose(0, 3, 2, 1)).astype(bf16),
        "b1d": c(b1.reshape(2 * n_ff, P).T),
        "b2d": c(b2.reshape(n_et, P).T),
        "lnv": c(np.concatenate([v.reshape(n_et, P).T for v in
                                 (ln1_w, ln1_b, ln2_w, ln2_b)], axis=1)),
    }
    in_maps = []
    for core in range(N_CORES):
        b, r = core // R, core % R
        order = [r] + [x for x in range(R) if x != r]
        blocks = [src[b, x * T_OWN:(x + 1) * T_OWN, :].T for x in order]
        m = dict(shared)
        m["srcT"] = c(np.concatenate(blocks, axis=1))
        in_maps.append(m)
    return in_maps


_CACHE = {}


def _compiled(cfg_key):
    if cfg_key not in _CACHE:
        E, T_OWN, FF, R = cfg_key
        nc = bacc.Bacc("TRN2", target_bir_lowering=False, debug=False,
                       num_devices=N_CORES)
        build(nc, E, T_OWN, FF, R)
        nc.compile()
        _CACHE[cfg_key] = nc
    return _CACHE[cfg_key]


def run(inputs, cfg, trace=False, tmpdir=None, trace_cores=None):
    E, T_OWN, R = cfg["E"], cfg["T_OWN"], cfg["R"]
    nc = _compiled((E, T_OWN, cfg["FF"], R))
    in_maps = prep_inputs(
        np.asarray(inputs["src"], np.float32),
        np.asarray(inputs["Wq"], np.float32), np.asarray(inputs["Wk"], np.float32),
        np.asarray(inputs["Wv"], np.float32), np.asarray(inputs["Wo"], np.float32),
        np.asarray(inputs["W1"], np.float32), np.asarray(inputs["b1"], np.float32),
        np.asarray(inputs["W2"], np.float32), np.asarray(inputs["b2"], np.float32),
        np.asarray(inputs["ln1_w"], np.float32), np.asarray(inputs["ln1_b"], np.float32),
        np.asarray(inputs["ln2_w"], np.float32), np.asarray(inputs["ln2_b"], np.float32),
        E, T_OWN, cfg["FF"], R)
    res = run_bass_kernel_spmd(nc, in_maps, core_ids=list(range(N_CORES)),
                               trace=trace, tmpdir=tmpdir, trace_cores=trace_cores)
    B, T = 8 // R, R * T_OWN
    out = np.empty((B, T, E), np.float32)
    for core in range(N_CORES):
        b, r = core // R, core % R
        out[b, r * T_OWN:(r + 1) * T_OWN, :] = res.results[core]["outT"].T
    return out, res


def kernel(**inputs) -> np.ndarray:
    out, _ = run(inputs, FULL)
    return out



# revision 2
# speedup vs baseline: 1.0190x; 1.0190x over previous
"""Trainium2 Bass kernel for a pre-norm transformer encoder layer with GEGLU FFN.

V3 sharding: token-data-parallel over 8 cores (core c: batch c//4, 512-token
chunk c%4). Each core computes LN1/Q/K/V only for its OWN 512 tokens; the
full-sequence K (feature-major) and V (token-major) are assembled with one
AllGather per 4-core group (HBM-HBM, runs on TOPSP/SDMA, overlapped with the
Q projection). All matmuls are bf16 at full 512-wide moving operands; softmax
runs on transposed scores with the denominator via a ones-column in the AV
matmul. No DRAM round-trips for activations.
"""

import numpy as np

import concourse.bass as bass
import concourse.mybir as mybir
import concourse.tile as tile
from concourse import bacc
from concourse.bass_utils import run_bass_kernel_spmd

P = 128
D = 64  # head dim (fixed)
F32 = mybir.dt.float32
F32R = mybir.dt.float32r
BF16 = mybir.dt.bfloat16
AF = mybir.ActivationFunctionType
ALU = mybir.AluOpType

FULL = dict(E=1024, T_OWN=512, FF=4096, R=4)
EPS = 1e-5
N_CORES = 8
GROUPS = [[0, 1, 2, 3], [4, 5, 6, 7]]


def build(nc, E, T_OWN, FF, R):
    H = E // D            # heads
    n_et = E // P         # feature tiles == head-pairs
    n_ff = FF // P        # ff tiles per GEGLU half
    T = R * T_OWN         # full sequence
    n_tt = T // P         # key tiles
    n_to = T_OWN // P     # own-token tiles
    T1 = T_OWN
    assert T1 <= 512
    OC_W = min(512, E)
    n_oc = E // OC_W

    # ---- DRAM I/O (own 512-token chunk only) ----
    srcT = nc.dram_tensor("srcT", [E, T1], F32R, kind="ExternalInput")
    wq = nc.dram_tensor("wq", [n_et, P, n_et, P], BF16, kind="ExternalInput")
    wk = nc.dram_tensor("wk", [n_et, P, n_et, P], BF16, kind="ExternalInput")
    wv = nc.dram_tensor("wv", [n_oc, n_et, P, OC_W], BF16, kind="ExternalInput")
    wo = nc.dram_tensor("wo", [n_et, P, n_et, P], BF16, kind="ExternalInput")
    w1 = nc.dram_tensor("w1", [2 * n_ff, P, n_et, P], BF16, kind="ExternalInput")
    w2 = nc.dram_tensor("w2", [n_et, P, n_ff, P], BF16, kind="ExternalInput")
    b1d = nc.dram_tensor("b1d", [P, 2 * n_ff], F32, kind="ExternalInput")
    b2d = nc.dram_tensor("b2d", [P, n_et], F32, kind="ExternalInput")
    lnv = nc.dram_tensor("lnv", [P, 4 * n_et], F32, kind="ExternalInput")
    outT = nc.dram_tensor("outT", [E, T1], F32, kind="ExternalOutput")

    def mm(ps, lhsT, rhs, start, stop):
        nc.tensor.matmul(ps, lhsT, rhs, start=start, stop=stop)

    with nc.allow_low_precision(reason="bf16 tiles feeding PE; fp32 PSUM accumulation"), \
            tile.TileContext(nc) as tc, tc.tile_pool(name="consts", bufs=1) as constp:
        def single(shape, name, dt=F32):
            return constp.tile(shape, dt, name=name, tag=name)

        ones_col = single([P, 1], "ones_col", F32R)
        nc.vector.memset(ones_col[:].bitcast(F32), 1.0)
        ones_row = single([1, P], "ones_row", F32R)
        nc.vector.memset(ones_row[:].bitcast(F32), 1.0)
        eps_t = single([1, 1], "eps_t")
        nc.vector.memset(eps_t[:], EPS)

        lnc = single([P, 4 * n_et], "lnc")
        nc.sync.dma_start(lnc[:], lnv[:])
        b1c = single([P, 2 * n_ff], "b1c")
        nc.sync.dma_start(b1c[:], b1d[:])
        b2c = single([P, n_et], "b2c")
        nc.sync.dma_start(b2c[:], b2d[:])

        from contextlib import ExitStack
        with ExitStack() as es:
            pool = lambda **kw: es.enter_context(tc.tile_pool(**kw))
            srcp = pool(name="srcp", bufs=n_et)          # resident own src (f32r)
            hTp = pool(name="hT", bufs=12)               # h1 / attnT / h2 ring
            qp = pool(name="qp", bufs=n_et)              # resident q bf16
            xp = pool(name="xp", bufs=n_et)              # resident x f32r
            bigp = pool(name="big", bufs=n_ff)           # resident f bf16
            wblkp = pool(name="wblk", bufs=6)            # wq/wo/w1 stream
            wkresp = pool(name="wkres", bufs=n_et)       # resident wk bf16
            wvresp = pool(name="wvres", bufs=n_oc * n_et)
            w2p = pool(name="w2p", bufs=4)
            kvevp = pool(name="kvev", bufs=4)            # K/V psum evictions
            kslabp = pool(name="kslab", bufs=3)          # [P,T] bf16 K stream
            vsbp = pool(name="vsb", bufs=5)              # [P,n_tt,D+1] bf16 V
            probsp = pool(name="probs", bufs=8)
            evp = pool(name="ev", bufs=6)
            rbp = pool(name="rb", bufs=2)
            statsp = pool(name="stats", bufs=6)
            psS = pool(name="psS", bufs=3, space="PSUM")
            psA = pool(name="psA", bufs=4, space="PSUM")
            psM = pool(name="psM", bufs=1, space="PSUM")
            dram = pool(name="dram", bufs=1, space="DRAM")

            _gp = [0]
            _gp_targets = [(psS, "psS"), (psA, "psA"), (psS, "psS"),
                           (psM, "psM"), (psA, "psA")]

            def group_psum(name, width=T1):
                pool_, tag = _gp_targets[_gp[0] % len(_gp_targets)]
                _gp[0] += 1
                return pool_.tile([P, width], F32, name=name, tag=tag)

            # collective bounce buffers (internal DRAM)
            k_cc_in = dram.tile([E, T1], BF16, name="k_cc_in")
            k_cc_out = dram.tile([R * E, T1], BF16, name="k_cc_out")
            v_cc_in = dram.tile([T1, E], BF16, name="v_cc_in")
            v_cc_out = dram.tile([T, E], BF16, name="v_cc_out")

            def layer_norm(x_tiles, wcol, bcol, label, out_pool, width):
                """x_tiles: n_et SBUF [P, width] feature-major; returns bf16."""
                m_ps = psS.tile([1, width], F32, name=f"mps_{label}", tag="psS")
                s_ps = psS.tile([1, width], F32, name=f"sps_{label}", tag="psS")
                for kt in range(n_et):
                    mm(m_ps[:], ones_col[:], x_tiles[kt][:], kt == 0, kt == n_et - 1)
                for kt in range(n_et):
                    sq = evp.tile([P, width], F32R, name=f"sq_{label}{kt}", tag="ev")
                    if kt % 2 == 0:
                        nc.scalar.square(sq[:], x_tiles[kt][:])
                    else:
                        nc.vector.tensor_mul(sq[:], x_tiles[kt][:], x_tiles[kt][:])
                    mm(s_ps[:], ones_col[:], sq[:], kt == 0, kt == n_et - 1)
                m_sb = statsp.tile([1, width], F32R, name=f"m_{label}", tag="st")
                nc.vector.tensor_scalar_mul(m_sb[:], m_ps[:], 1.0 / E)
                msq = statsp.tile([1, width], F32, name=f"msq_{label}", tag="st")
                nc.vector.tensor_mul(msq[:], m_sb[:], m_sb[:])
                var = statsp.tile([1, width], F32, name=f"var_{label}", tag="st")
                nc.vector.scalar_tensor_tensor(
                    var[:], s_ps[:], 1.0 / E, msq[:], ALU.mult, ALU.subtract)
                sd = statsp.tile([1, width], F32, name=f"sd_{label}", tag="st")
                nc.scalar.activation(sd[:], var[:], AF.Sqrt, bias=eps_t[0:1, 0:1])
                rinv = statsp.tile([1, width], F32R, name=f"ri_{label}", tag="st")
                nc.vector.reciprocal(rinv[:], sd[:])
                mr_sb = statsp.tile([1, width], F32R, name=f"mr_{label}", tag="st")
                nc.vector.tensor_mul(mr_sb[:], m_sb[:], rinv[:])
                rs_ps = psS.tile([P, width], F32, name=f"rb_{label}", tag="psS")
                mm(rs_ps[:], ones_row[0:1, :], rinv[:], True, True)
                mr_ps = psM.tile([P, width], F32, name=f"mb_{label}", tag="psM")
                mm(mr_ps[:], ones_row[0:1, :], mr_sb[:], True, True)
                rs_sb = evp.tile([P, width], F32, name=f"rsb_{label}", tag="ev")
                nc.vector.tensor_copy(rs_sb[:], rs_ps[:])
                mrb_sb = evp.tile([P, width], F32, name=f"mrb_{label}", tag="ev")
                nc.vector.tensor_copy(mrb_sb[:], mr_ps[:])
                h_tiles = []
                for kt in range(n_et):
                    t = evp.tile([P, width], F32, name=f"xc_{label}{kt}", tag="ev")
                    nc.vector.tensor_mul(t[:], x_tiles[kt][:], rs_sb[:])
                    t2 = evp.tile([P, width], F32, name=f"xs_{label}{kt}", tag="ev")
                    nc.vector.tensor_sub(t2[:], t[:], mrb_sb[:])
                    h = out_pool.tile([P, width], BF16, name=f"h_{label}{kt}", tag="hT")
                    nc.scalar.activation(h[:], t2[:], AF.Identity,
                                         bias=bcol[:, kt:kt + 1],
                                         scale=wcol[:, kt:kt + 1])
                    h_tiles.append(h)
                return h_tiles

            # ---- prologue DMAs: src chunk first, then resident K/V weights ----
            src_sb = []
            for kt in range(n_et):
                sx = srcp.tile([P, T1], F32R, name=f"src_{kt}", tag="src")
                nc.sync.dma_start(sx[:], srcT[kt * P:(kt + 1) * P, :])
                src_sb.append(sx)
            wk_res = []
            for ot in range(n_et):
                wkr = wkresp.tile([P, n_et, P], BF16, name=f"wkr{ot}", tag="wkr")
                nc.sync.dma_start(wkr[:], wk[ot])
                wk_res.append(wkr)
            wv_res = {}
            for oc in range(n_oc):
                for kt in range(n_et):
                    wvr = wvresp.tile([P, OC_W], BF16, name=f"wvr{oc}_{kt}", tag="wvr")
                    nc.sync.dma_start(wvr[:], wv[oc, kt])
                    wv_res[(oc, kt)] = wvr

            # ---- LN1 on own chunk ----
            with nc.named_scope("ln1"):
                h1 = layer_norm(src_sb, lnc[:, 0:n_et], lnc[:, n_et:2 * n_et],
                                "l1", hTp, T1)

            # ---- K own -> bounce -> AllGather ----
            with nc.named_scope("kproj"):
                for ot in range(n_et):
                    ps = group_psum(f"psk{ot}")
                    for kt in range(n_et):
                        mm(ps[:], wk_res[ot][:, kt, :], h1[kt][:], kt == 0, kt == n_et - 1)
                    kev = kvevp.tile([P, T1], BF16, name=f"kev{ot}", tag="kvev")
                    nc.scalar.copy(kev[:], ps[:])
                    nc.sync.dma_start(k_cc_in[ot * P:(ot + 1) * P, :], kev[:])
                nc.gpsimd.collective_compute(
                    "AllGather", ALU.bypass, replica_groups=GROUPS,
                    ins=[k_cc_in.opt()], outs=[k_cc_out.opt()])

            # ---- V own (token-major) -> bounce -> AllGather ----
            with nc.named_scope("vproj"):
                for oc in range(n_oc):
                    for to in range(n_to):
                        ps = group_psum(f"psv{oc}_{to}", OC_W)
                        for kt in range(n_et):
                            mm(ps[:], h1[kt][:, to * P:(to + 1) * P], wv_res[(oc, kt)][:],
                               kt == 0, kt == n_et - 1)
                        vev = kvevp.tile([P, OC_W], BF16, name=f"vev{oc}_{to}", tag="kvev")
                        nc.vector.tensor_copy(vev[:], ps[:])
                        nc.sync.dma_start(
                            v_cc_in[to * P:(to + 1) * P, oc * OC_W:(oc + 1) * OC_W],
                            vev[:])
                nc.gpsimd.collective_compute(
                    "AllGather", ALU.bypass, replica_groups=GROUPS,
                    ins=[v_cc_in.opt()], outs=[v_cc_out.opt()])

            # ---- Q own (overlaps the AllGathers) ----
            q_sb = []
            with nc.named_scope("qproj"):
                for ot in range(n_et):
                    wq_sb = wblkp.tile([P, n_et, P], BF16, name=f"wq{ot}", tag="wblk")
                    nc.sync.dma_start(wq_sb[:], wq[ot])
                    ps = group_psum(f"psq{ot}")
                    for kt in range(n_et):
                        mm(ps[:], wq_sb[:, kt, :], h1[kt][:], kt == 0, kt == n_et - 1)
                    q = qp.tile([P, T1], BF16, name=f"q{ot}", tag="q")
                    nc.vector.tensor_copy(q[:], ps[:])
                    q_sb.append(q)

            # ---- attention, one head-pair at a time; K/V streamed from AG out ----
            with nc.named_scope("attn"):
                attn_sb = []

                def emit_normalize(a, att_ps, hp):
                    recs, rbpss, rbs = [], [], []
                    for hl in range(2):
                        rec = statsp.tile([1, T1], F32R, name=f"rec{hp}_{hl}", tag="st")
                        nc.vector.reciprocal(rec[:], att_ps[hl][D:D + 1, :])
                        recs.append(rec)
                    for hl in range(2):
                        rbps = psM.tile([P, T1], F32, name=f"rbp{hp}_{hl}", tag="psM")
                        mm(rbps[0:D, :], ones_row[0:1, 0:D], recs[hl][:], True, True)
                        rbpss.append(rbps)
                    for hl in range(2):
                        rb_sb = rbp.tile([D, T1], F32, name=f"rbs{hp}_{hl}", tag="rb")
                        nc.vector.tensor_copy(rb_sb[:], rbpss[hl][0:D, :])
                        rbs.append(rb_sb)
                    for hl in range(2):
                        nc.vector.tensor_mul(a[hl * D:(hl + 1) * D, :],
                                             att_ps[hl][0:D, :], rbs[hl][:])

                pending = None
                for hp in range(n_et):
                    kslab = kslabp.tile([P, T], BF16, name=f"ks{hp}", tag="ks")
                    for rk in range(R):
                        nc.sync.dma_start(
                            kslab[:, rk * T1:(rk + 1) * T1],
                            k_cc_out[rk * E + hp * P: rk * E + (hp + 1) * P, :])
                    vsb = []
                    for hl in range(2):
                        h_idx = hp * 2 + hl
                        v = vsbp.tile([P, n_tt, D + 1], BF16, name=f"v{hp}_{hl}", tag="vs")
                        nc.sync.dma_start(
                            v[:, :, 0:D],
                            v_cc_out[:, h_idx * D:(h_idx + 1) * D]
                            .rearrange("(tt p) d -> p tt d", p=P))
                        nc.gpsimd.memset(v[:, :, D:D + 1], 1.0)
                        vsb.append(v)
                    att_ps = [psA.tile([D + 1, T1], F32, name=f"pa{hp}_{hl}", tag="psA")
                              for hl in range(2)]
                    for tt in range(n_tt):
                        for hl in range(2):
                            sc = psS.tile([P, T1], F32, name=f"sc{hp}_{tt}_{hl}", tag="psS")
                            mm(sc[:],
                               kslab[hl * D:(hl + 1) * D, tt * P:(tt + 1) * P],
                               q_sb[hp][hl * D:(hl + 1) * D, :], True, True)
                            pr = probsp.tile([P, T1], BF16, name=f"pr{hp}_{tt}_{hl}", tag="pr")
                            nc.scalar.activation(pr[:], sc[:], AF.Exp, scale=0.125)
                            if tt == 10 and hl == 0 and pending is not None:
                                emit_normalize(*pending)
                                pending = None
                            mm(att_ps[hl][:], vsb[hl][:, tt, :], pr[:],
                               tt == 0, tt == n_tt - 1)
                    a = hTp.tile([P, T1], BF16, name=f"attn{hp}", tag="hT")
                    attn_sb.append(a)
                    pending = (a, att_ps, hp)
                emit_normalize(*pending)

            # ---- Wo + residual -> x ----
            with nc.named_scope("wo"):
                x_sb = []
                for ot in range(n_et):
                    wo_sb = wblkp.tile([P, n_et, P], BF16, name=f"wo{ot}", tag="wblk")
                    nc.sync.dma_start(wo_sb[:], wo[ot])
                    ps = group_psum(f"pso{ot}")
                    for kt in range(n_et):
                        mm(ps[:], wo_sb[:, kt, :], attn_sb[kt][:], kt == 0, kt == n_et - 1)
                    x = xp.tile([P, T1], F32R, name=f"x{ot}", tag="x")
                    nc.vector.tensor_add(x[:], ps[:], src_sb[ot][:])
                    x_sb.append(x)

            # ---- LN2 ----
            with nc.named_scope("ln2"):
                h2 = layer_norm(x_sb, lnc[:, 2 * n_et:3 * n_et],
                                lnc[:, 3 * n_et:4 * n_et], "l2", hTp, T1)

            # ---- FFN1: GEGLU ----
            with nc.named_scope("ffn1"):
                f_sb = []
                for pt in range(n_ff):
                    w1a = wblkp.tile([P, n_et, P], BF16, name=f"w1a{pt}", tag="wblk")
                    nc.sync.dma_start(w1a[:], w1[pt])
                    w1g = wblkp.tile([P, n_et, P], BF16, name=f"w1g{pt}", tag="wblk")
                    nc.sync.dma_start(w1g[:], w1[n_ff + pt])
                    psa = group_psum(f"psa{pt}")
                    for kt in range(n_et):
                        mm(psa[:], w1a[:, kt, :], h2[kt][:], kt == 0, kt == n_et - 1)
                    psg = group_psum(f"psg{pt}")
                    for kt in range(n_et):
                        mm(psg[:], w1g[:, kt, :], h2[kt][:], kt == 0, kt == n_et - 1)
                    gel = evp.tile([P, T1], F32, name=f"gel{pt}", tag="ev")
                    nc.scalar.activation(gel[:], psg[:], AF.Gelu,
                                         bias=b1c[:, n_ff + pt:n_ff + pt + 1])
                    f = bigp.tile([P, T1], BF16, name=f"f{pt}", tag="big")
                    nc.vector.scalar_tensor_tensor(
                        f[:], psa[:], b1c[:, pt:pt + 1], gel[:], ALU.add, ALU.mult)
                    f_sb.append(f)

            # ---- W2 + b2 + residual -> outT ----
            with nc.named_scope("w2out"):
                n_ffh = max(1, n_ff // 4)
                for ot in range(n_et):
                    ps = group_psum(f"psy{ot}")
                    w2h = []
                    for half in range(n_ff // n_ffh):
                        w = w2p.tile([P, n_ffh, P], BF16, name=f"w2_{ot}_{half}", tag="w2")
                        nc.sync.dma_start(
                            w[:], w2[ot, :, half * n_ffh:(half + 1) * n_ffh, :])
                        w2h.append(w)
                    for c in range(n_ff):
                        mm(ps[:], w2h[c // n_ffh][:, c % n_ffh, :], f_sb[c][:],
                           c == 0, c == n_ff - 1)
                    y = evp.tile([P, T1], F32, name=f"y{ot}", tag="ev")
                    nc.vector.scalar_tensor_tensor(
                        y[:], ps[:], b2c[:, ot:ot + 1], x_sb[ot][:], ALU.add, ALU.add)
                    nc.sync.dma_start(outT[ot * P:(ot + 1) * P, :], y[:])

    return nc


def prep_inputs(src, Wq, Wk, Wv, Wo, W1, b1, W2, b2,
                ln1_w, ln1_b, ln2_w, ln2_b, E, T_OWN, FF, R):
    """Host-side: transpose/retile weights, shard src. Returns per-core in_maps."""
    n_et = E // P
    n_ff = FF // P
    OC_W = min(512, E)
    n_oc = E // OC_W
    import ml_dtypes
    bf16 = ml_dtypes.bfloat16
    c = np.ascontiguousarray
    shared = {
        "wq": c(Wq.reshape(n_et, P, n_et, P).transpose(0, 3, 2, 1)).astype(bf16),
        "wk": c(Wk.reshape(n_et, P, n_et, P).transpose(0, 3, 2, 1)).astype(bf16),
        "wv": c(Wv.reshape(n_oc, OC_W, n_et, P).transpose(0, 2, 3, 1)).astype(bf16),
        "wo": c(Wo.reshape(n_et, P, n_et, P).transpose(0, 3, 2, 1)).astype(bf16),
        "w1": c(W1.reshape(2 * n_ff, P, n_et, P).transpose(0, 3, 2, 1)).astype(bf16),
        "w2": c(W2.reshape(n_et, P, n_ff, P).transpose(0, 3, 2, 1)).astype(bf16),
        "b1d": c(b1.reshape(2 * n_ff, P).T),
        "b2d": c(b2.reshape(n_et, P).T),
        "lnv": c(np.concatenate([v.reshape(n_et, P).T for v in
                                 (ln1_w, ln1_b, ln2_w, ln2_b)], axis=1)),
    }
    in_maps = []
    for core in range(N_CORES):
        b, r = core // R, core % R
        m = dict(shared)
        m["srcT"] = c(src[b, r * T_OWN:(r + 1) * T_OWN, :].T)
        in_maps.append(m)
    return in_maps


_CACHE = {}


def _compiled(cfg_key):
    if cfg_key not in _CACHE:
        E, T_OWN, FF, R = cfg_key
        nc = bacc.Bacc("TRN2", target_bir_lowering=False, debug=False,
                       num_devices=N_CORES)
        build(nc, E, T_OWN, FF, R)
        nc.compile()
        _CACHE[cfg_key] = nc
    return _CACHE[cfg_key]


def run(inputs, cfg, trace=False, tmpdir=None, trace_cores=None):
    E, T_OWN, R = cfg["E"], cfg["T_OWN"], cfg["R"]
    nc = _compiled((E, T_OWN, cfg["FF"], R))
    in_maps = prep_inputs(
        np.asarray(inputs["src"], np.float32),
        np.asarray(inputs["Wq"], np.float32), np.asarray(inputs["Wk"], np.float32),
        np.asarray(inputs["Wv"], np.float32), np.asarray(inputs["Wo"], np.float32),
        np.asarray(inputs["W1"], np.float32), np.asarray(inputs["b1"], np.float32),
        np.asarray(inputs["W2"], np.float32), np.asarray(inputs["b2"], np.float32),
        np.asarray(inputs["ln1_w"], np.float32), np.asarray(inputs["ln1_b"], np.float32),
        np.asarray(inputs["ln2_w"], np.float32), np.asarray(inputs["ln2_b"], np.float32),
        E, T_OWN, cfg["FF"], R)
    res = run_bass_kernel_spmd(nc, in_maps, core_ids=list(range(N_CORES)),
                               trace=trace, tmpdir=tmpdir, trace_cores=trace_cores)
    B, T = 8 // R, R * T_OWN
    out = np.empty((B, T, E), np.float32)
    for core in range(N_CORES):
        b, r = core // R, core % R
        out[b, r * T_OWN:(r + 1) * T_OWN, :] = res.results[core]["outT"].T
    return out, res


def kernel(**inputs) -> np.ndarray:
    out, _ = run(inputs, FULL)
    return out


# revision 13
# speedup vs baseline: 1.1510x; 1.1296x over previous
"""Trainium2 Bass kernel for a pre-norm transformer encoder layer with GEGLU FFN.

V4 sharding: token-data-parallel over 8 cores (core c: batch c//4, 512-token
chunk c%4). Each core computes LN1/Q/K/V only for its OWN 512 tokens; the
full-sequence K (feature-major) and V (token-major) are exchanged in fp8 with
one AllGather each per 4-core group (HBM-HBM on TOPSP/SDMA, overlapped with
the V/Q projections). Attention is software-pipelined: both heads' scores of
a key-tile land in one two-bank PSUM tile, a single wide Exp activation
converts them to fp8 probs, and the AV matmuls lag one key-tile behind so the
scalar-engine exp latency never stalls the in-order PE queue. Softmax/LN
broadcasts run on the idle GPSIMD engine instead of PE matmuls.
"""

import numpy as np

import concourse.bass as bass
import concourse.mybir as mybir
import concourse.tile as tile
from concourse import bacc
from concourse.bass_utils import run_bass_kernel_spmd

P = 128
D = 64  # head dim (fixed)
F32 = mybir.dt.float32
F32R = mybir.dt.float32r
BF16 = mybir.dt.bfloat16
FP8 = mybir.dt.float8e4
AF = mybir.ActivationFunctionType
ALU = mybir.AluOpType

FULL = dict(E=1024, T_OWN=512, FF=4096, R=4)
EPS = 1e-5
N_CORES = 8
GROUPS = [[0, 1, 2, 3], [4, 5, 6, 7]]


def build(nc, E, T_OWN, FF, R):
    H = E // D            # heads
    n_et = E // P         # feature tiles == head-pairs
    n_ff = FF // P        # ff tiles per GEGLU half
    T = R * T_OWN         # full sequence
    n_tt = T // P         # key tiles
    n_to = T_OWN // P     # own-token tiles
    T1 = T_OWN
    assert T1 <= 512
    OC_W = min(512, E)
    n_oc = E // OC_W

    # ---- DRAM I/O (own 512-token chunk only) ----
    srcT = nc.dram_tensor("srcT", [E, T1], F32R, kind="ExternalInput")
    wq = nc.dram_tensor("wq", [n_et, P, n_et, P], BF16, kind="ExternalInput")
    wk = nc.dram_tensor("wk", [n_et, P, n_et, P], BF16, kind="ExternalInput")
    wv = nc.dram_tensor("wv", [n_oc, n_et, P, OC_W], BF16, kind="ExternalInput")
    wo = nc.dram_tensor("wo", [n_et, P, n_et, P], BF16, kind="ExternalInput")
    w1 = nc.dram_tensor("w1", [2 * n_ff, P, n_et, P], BF16, kind="ExternalInput")
    w2 = nc.dram_tensor("w2", [n_et, P, n_ff, P], BF16, kind="ExternalInput")
    b1d = nc.dram_tensor("b1d", [P, 2 * n_ff], F32, kind="ExternalInput")
    b2d = nc.dram_tensor("b2d", [P, n_et], F32, kind="ExternalInput")
    lnv = nc.dram_tensor("lnv", [P, 4 * n_et], F32, kind="ExternalInput")
    outT = nc.dram_tensor("outT", [E, T1], F32, kind="ExternalOutput")

    def mm(ps, lhsT, rhs, start, stop):
        nc.tensor.matmul(ps, lhsT, rhs, start=start, stop=stop)

    with nc.allow_low_precision(reason="bf16/fp8 tiles feeding PE; fp32 PSUM accumulation"), \
            tile.TileContext(nc) as tc, tc.tile_pool(name="consts", bufs=1) as constp:
        def single(shape, name, dt=F32):
            return constp.tile(shape, dt, name=name, tag=name)

        ones_col = single([P, 1], "ones_col", F32R)
        nc.vector.memset(ones_col[:].bitcast(F32), 1.0)
        ones_row = single([1, P], "ones_row", F32R)
        nc.vector.memset(ones_row[:].bitcast(F32), 1.0)
        eps_t = single([1, 1], "eps_t")
        nc.vector.memset(eps_t[:], EPS)
        nb2_t = single([P, 1], "nb2_t")
        nc.vector.memset(nb2_t[:], -2.0)

        lnc = single([P, 4 * n_et], "lnc")
        nc.sync.dma_start(lnc[:], lnv[:])
        b1c = single([P, 2 * n_ff], "b1c")
        nc.sync.dma_start(b1c[:], b1d[:])
        b2c = single([P, n_et], "b2c")
        nc.sync.dma_start(b2c[:], b2d[:])

        from contextlib import ExitStack
        with ExitStack() as es:
            pool = lambda **kw: es.enter_context(tc.tile_pool(**kw))
            srcp = pool(name="srcp", bufs=n_et)          # resident own src (f32r)
            hTp = pool(name="hT", bufs=12)               # h1 / attnT / h2 ring
            qp = pool(name="qp", bufs=n_et)              # resident q fp8
            xp = pool(name="xp", bufs=n_et)              # resident x f32r
            bigp = pool(name="big", bufs=n_ff)           # resident f bf16
            wblkp = pool(name="wblk", bufs=8)            # wq/wo/w1 stream
            wkresp = pool(name="wkres", bufs=n_et)       # resident wk bf16
            wvresp = pool(name="wvres", bufs=n_oc * n_et)
            w2p = pool(name="w2p", bufs=6)
            kvevp = pool(name="kvev", bufs=4)            # K/V psum evictions fp8
            kslabp = pool(name="kslab", bufs=3)          # [P,T] fp8 K stream
            vsbp = pool(name="vsb", bufs=5)              # [P,n_tt,D+1] fp8 V
            probsp = pool(name="probs", bufs=6)          # [P,2*T1] fp8
            evp = pool(name="ev", bufs=6)
            rbp = pool(name="rb", bufs=4)
            statsp = pool(name="stats", bufs=8)
            psS = pool(name="psS", bufs=4, space="PSUM")  # [P, T1] score tiles
            psA = pool(name="psA", bufs=4, space="PSUM")  # [P, T1] accum/groups
            dram = pool(name="dram", bufs=1, space="DRAM")

            def group_psum(name, width=T1):
                # accumulation groups rotate over the psA ring (4 banks)
                return psA.tile([P, width], F32, name=name, tag="psA")

            # collective bounce buffers (internal DRAM, fp8)
            k_cc_in = dram.tile([E, T1], FP8, name="k_cc_in")
            k_cc_out = dram.tile([R * E, T1], FP8, name="k_cc_out")
            v_cc_in = dram.tile([T1, E], FP8, name="v_cc_in")
            v_cc_out = dram.tile([T, E], FP8, name="v_cc_out")

            def bcast(out_ap, in_ap):
                nc.gpsimd.partition_broadcast(out_ap, in_ap)

            def layer_norm(x_tiles, wcol, bcol, label, out_pool, width):
                """x_tiles: n_et SBUF [P, width] feature-major; returns bf16."""
                m_ps = psA.tile([1, width], F32, name=f"mps_{label}", tag="psA")
                s_ps = psA.tile([1, width], F32, name=f"sps_{label}", tag="psA")
                for kt in range(n_et):
                    mm(m_ps[:], ones_col[:], x_tiles[kt][:], kt == 0, kt == n_et - 1)
                for kt in range(n_et):
                    sq = evp.tile([P, width], F32R, name=f"sq_{label}{kt}", tag="ev")
                    if kt % 2 == 0:
                        nc.scalar.square(sq[:], x_tiles[kt][:])
                    else:
                        nc.vector.tensor_mul(sq[:], x_tiles[kt][:], x_tiles[kt][:])
                    mm(s_ps[:], ones_col[:], sq[:], kt == 0, kt == n_et - 1)
                m_sb = statsp.tile([1, width], F32R, name=f"m_{label}", tag="st")
                nc.vector.tensor_scalar_mul(m_sb[:], m_ps[:], 1.0 / E)
                msq = statsp.tile([1, width], F32, name=f"msq_{label}", tag="st")
                nc.vector.tensor_mul(msq[:], m_sb[:], m_sb[:])
                var = statsp.tile([1, width], F32, name=f"var_{label}", tag="st")
                nc.vector.scalar_tensor_tensor(
                    var[:], s_ps[:], 1.0 / E, msq[:], ALU.mult, ALU.subtract)
                sd = statsp.tile([1, width], F32, name=f"sd_{label}", tag="st")
                nc.scalar.activation(sd[:], var[:], AF.Sqrt, bias=eps_t[0:1, 0:1])
                rinv = statsp.tile([1, width], F32, name=f"ri_{label}", tag="st")
                nc.vector.reciprocal(rinv[:], sd[:])
                mr_sb = statsp.tile([1, width], F32, name=f"mr_{label}", tag="st")
                nc.vector.tensor_mul(mr_sb[:], m_sb[:], rinv[:])
                rs_sb = evp.tile([P, width], F32, name=f"rsb_{label}", tag="ev")
                bcast(rs_sb[:], rinv[:])
                mrb_sb = evp.tile([P, width], F32, name=f"mrb_{label}", tag="ev")
                bcast(mrb_sb[:], mr_sb[:])
                h_tiles = []
                for kt in range(n_et):
                    t = evp.tile([P, width], F32, name=f"xc_{label}{kt}", tag="ev")
                    nc.vector.tensor_mul(t[:], x_tiles[kt][:], rs_sb[:])
                    t2 = evp.tile([P, width], F32, name=f"xs_{label}{kt}", tag="ev")
                    nc.vector.tensor_sub(t2[:], t[:], mrb_sb[:])
                    h = out_pool.tile([P, width], BF16, name=f"h_{label}{kt}", tag="hT")
                    nc.scalar.activation(h[:], t2[:], AF.Identity,
                                         bias=bcol[:, kt:kt + 1],
                                         scale=wcol[:, kt:kt + 1])
                    h_tiles.append(h)
                return h_tiles

            # ---- prologue DMAs ----
            src_sb = []
            for kt in range(n_et):
                sx = srcp.tile([P, T1], F32R, name=f"src_{kt}", tag="src")
                nc.sync.dma_start(sx[:], srcT[kt * P:(kt + 1) * P, :])
                src_sb.append(sx)
            wk_res = []
            for ot in range(n_et):
                wkr = wkresp.tile([P, n_et, P], BF16, name=f"wkr{ot}", tag="wkr")
                nc.sync.dma_start(wkr[:], wk[ot])
                wk_res.append(wkr)
            wv_res = {}
            for oc in range(n_oc):
                for kt in range(n_et):
                    wvr = wvresp.tile([P, OC_W], BF16, name=f"wvr{oc}_{kt}", tag="wvr")
                    nc.sync.dma_start(wvr[:], wv[oc, kt])
                    wv_res[(oc, kt)] = wvr
            wq_sb = []
            for ot in range(n_et):
                w_ = wblkp.tile([P, n_et, P], BF16, name=f"wq{ot}", tag="wblk")
                nc.sync.dma_start(w_[:], wq[ot])
                wq_sb.append(w_)

            # ---- LN1 on own chunk ----
            with nc.named_scope("ln1"):
                h1 = layer_norm(src_sb, lnc[:, 0:n_et], lnc[:, n_et:2 * n_et],
                                "l1", hTp, T1)

            # ---- K own -> fp8 bounce -> AllGather (on gpsimd queue) ----
            with nc.named_scope("kproj"):
                for ot in range(n_et):
                    ps = group_psum(f"psk{ot}")
                    for kt in range(n_et):
                        mm(ps[:], wk_res[ot][:, kt, :], h1[kt][:], kt == 0, kt == n_et - 1)
                    kev = kvevp.tile([P, T1], FP8, name=f"kev{ot}", tag="kvev")
                    nc.scalar.copy(kev[:], ps[:])
                    nc.gpsimd.dma_start(k_cc_in[ot * P:(ot + 1) * P, :], kev[:])
                nc.gpsimd.collective_compute(
                    "AllGather", ALU.bypass, replica_groups=GROUPS,
                    ins=[k_cc_in.opt()], outs=[k_cc_out.opt()])

            # ---- V own (token-major) -> fp8 bounce -> AllGather ----
            with nc.named_scope("vproj"):
                for oc in range(n_oc):
                    for to in range(n_to):
                        ps = group_psum(f"psv{oc}_{to}", OC_W)
                        for kt in range(n_et):
                            mm(ps[:], h1[kt][:, to * P:(to + 1) * P], wv_res[(oc, kt)][:],
                               kt == 0, kt == n_et - 1)
                        vev = kvevp.tile([P, OC_W], FP8, name=f"vev{oc}_{to}", tag="kvev")
                        nc.vector.tensor_copy(vev[:], ps[:])
                        nc.gpsimd.dma_start(
                            v_cc_in[to * P:(to + 1) * P, oc * OC_W:(oc + 1) * OC_W],
                            vev[:])
                nc.gpsimd.collective_compute(
                    "AllGather", ALU.bypass, replica_groups=GROUPS,
                    ins=[v_cc_in.opt()], outs=[v_cc_out.opt()])

            # ---- Q own (overlaps the AllGathers) ----
            q_sb = []
            with nc.named_scope("qproj"):
                for ot in range(n_et):
                    ps = group_psum(f"psq{ot}")
                    for kt in range(n_et):
                        mm(ps[:], wq_sb[ot][:, kt, :], h1[kt][:], kt == 0, kt == n_et - 1)
                    q = qp.tile([P, T1], FP8, name=f"q{ot}", tag="q")
                    nc.vector.tensor_copy(q[:], ps[:])
                    q_sb.append(q)

            # ---- attention: pipelined scores->exp->AV, one head-pair at a time ----
            with nc.named_scope("attn"):
                attn_sb = []

                def emit_normalize(a, att_ps, hp):
                    recs = []
                    for hl in range(2):
                        rec = statsp.tile([1, T1], F32R, name=f"rec{hp}_{hl}", tag="st")
                        nc.vector.reciprocal(rec[:], att_ps[hl][D:D + 1, :])
                        recs.append(rec)
                    rbpss = []
                    for hl in range(2):
                        rbps = psS.tile([P, T1], F32, name=f"rbp{hp}_{hl}", tag="psS")
                        mm(rbps[0:D, :], ones_row[0:1, 0:D], recs[hl][:], True, True)
                        rbpss.append(rbps)
                    rbs = []
                    for hl in range(2):
                        rb_sb = rbp.tile([D, T1], F32, name=f"rbs{hp}_{hl}", tag="rb")
                        nc.vector.tensor_copy(rb_sb[:], rbpss[hl][0:D, :])
                        rbs.append(rb_sb)
                    for hl in range(2):
                        nc.vector.tensor_mul(a[hl * D:(hl + 1) * D, :],
                                             att_ps[hl][0:D, :], rbs[hl][:])

                pending = None
                for hp in range(n_et):
                    kslab = kslabp.tile([P, T], FP8, name=f"ks{hp}", tag="ks")
                    for rk in range(R):
                        nc.sync.dma_start(
                            kslab[:, rk * T1:(rk + 1) * T1],
                            k_cc_out[rk * E + hp * P: rk * E + (hp + 1) * P, :])
                    vsb = []
                    for hl in range(2):
                        h_idx = hp * 2 + hl
                        # D+16 stride keeps the 1-byte fp8 rows 16B-aligned
                        v = vsbp.tile([P, n_tt, D + 16], FP8, name=f"v{hp}_{hl}", tag="vs")
                        nc.sync.dma_start(
                            v[:, :, 0:D],
                            v_cc_out[:, h_idx * D:(h_idx + 1) * D]
                            .rearrange("(tt p) d -> p tt d", p=P))
                        nc.gpsimd.memset(v[:, :, D:D + 16], 1.0)
                        vsb.append(v)
                    att_ps = [psA.tile([P, T1], F32, name=f"pa{hp}_{hl}", tag="psA")
                              for hl in range(2)]
                    prev_pr = None
                    for tt in range(n_tt):
                        prs = []
                        for hl in range(2):
                            sc = psS.tile([P, T1], F32, name=f"sc{hp}_{tt}_{hl}", tag="psS")
                            mm(sc[:],
                               kslab[hl * D:(hl + 1) * D, tt * P:(tt + 1) * P],
                               q_sb[hp][hl * D:(hl + 1) * D, :], True, True)
                            pr = probsp.tile([P, T1], FP8, name=f"pr{hp}_{tt}_{hl}", tag="pr")
                            # bias shifts probs into fp8 range; softmax shift-invariant
                            nc.scalar.activation(pr[:], sc[:], AF.Exp,
                                                 scale=0.125, bias=nb2_t[:, 0:1])
                            prs.append(pr)
                        if tt == 6 and pending is not None:
                            emit_normalize(*pending)
                            pending = None
                        if prev_pr is not None:
                            ptt = tt - 1
                            for hl in range(2):
                                mm(att_ps[hl][0:D + 1, :], vsb[hl][:, ptt, 0:D + 1],
                                   prev_pr[hl][:], ptt == 0, False)
                        prev_pr = prs
                    for hl in range(2):
                        mm(att_ps[hl][0:D + 1, :], vsb[hl][:, n_tt - 1, 0:D + 1],
                           prev_pr[hl][:], False, True)
                    a = hTp.tile([P, T1], BF16, name=f"attn{hp}", tag="hT")
                    attn_sb.append(a)
                    pending = (a, att_ps, hp)
                emit_normalize(*pending)

            # ---- Wo + residual -> x ----
            with nc.named_scope("wo"):
                x_sb = []
                for ot in range(n_et):
                    wo_sb = wblkp.tile([P, n_et, P], BF16, name=f"wo{ot}", tag="wblk")
                    nc.sync.dma_start(wo_sb[:], wo[ot])
                    ps = group_psum(f"pso{ot}")
                    for kt in range(n_et):
                        mm(ps[:], wo_sb[:, kt, :], attn_sb[kt][:], kt == 0, kt == n_et - 1)
                    x = xp.tile([P, T1], F32R, name=f"x{ot}", tag="x")
                    nc.vector.tensor_add(x[:], ps[:], src_sb[ot][:])
                    x_sb.append(x)

            # ---- LN2 ----
            with nc.named_scope("ln2"):
                h2 = layer_norm(x_sb, lnc[:, 2 * n_et:3 * n_et],
                                lnc[:, 3 * n_et:4 * n_et], "l2", hTp, T1)

            # ---- FFN1: GEGLU ----
            with nc.named_scope("ffn1"):
                f_sb = []
                for pt in range(n_ff):
                    w1a = wblkp.tile([P, n_et, P], BF16, name=f"w1a{pt}", tag="wblk")
                    nc.sync.dma_start(w1a[:], w1[pt])
                    w1g = wblkp.tile([P, n_et, P], BF16, name=f"w1g{pt}", tag="wblk")
                    nc.sync.dma_start(w1g[:], w1[n_ff + pt])
                    psa = group_psum(f"psa{pt}")
                    for kt in range(n_et):
                        mm(psa[:], w1a[:, kt, :], h2[kt][:], kt == 0, kt == n_et - 1)
                    psg = group_psum(f"psg{pt}")
                    for kt in range(n_et):
                        mm(psg[:], w1g[:, kt, :], h2[kt][:], kt == 0, kt == n_et - 1)
                    gel = evp.tile([P, T1], F32, name=f"gel{pt}", tag="ev")
                    nc.scalar.activation(gel[:], psg[:], AF.Gelu,
                                         bias=b1c[:, n_ff + pt:n_ff + pt + 1])
                    f = bigp.tile([P, T1], BF16, name=f"f{pt}", tag="big")
                    nc.vector.scalar_tensor_tensor(
                        f[:], psa[:], b1c[:, pt:pt + 1], gel[:], ALU.add, ALU.mult)
                    f_sb.append(f)

            # ---- W2 + b2 + residual -> outT ----
            with nc.named_scope("w2out"):
                n_ffh = max(1, n_ff // 4)
                for ot in range(n_et):
                    ps = group_psum(f"psy{ot}")
                    w2h = []
                    for half in range(n_ff // n_ffh):
                        w = w2p.tile([P, n_ffh, P], BF16, name=f"w2_{ot}_{half}", tag="w2")
                        nc.sync.dma_start(
                            w[:], w2[ot, :, half * n_ffh:(half + 1) * n_ffh, :])
                        w2h.append(w)
                    for c in range(n_ff):
                        mm(ps[:], w2h[c // n_ffh][:, c % n_ffh, :], f_sb[c][:],
                           c == 0, c == n_ff - 1)
                    y = evp.tile([P, T1], F32, name=f"y{ot}", tag="ev")
                    nc.vector.scalar_tensor_tensor(
                        y[:], ps[:], b2c[:, ot:ot + 1], x_sb[ot][:], ALU.add, ALU.add)
                    nc.sync.dma_start(outT[ot * P:(ot + 1) * P, :], y[:])

    return nc


def prep_inputs(src, Wq, Wk, Wv, Wo, W1, b1, W2, b2,
                ln1_w, ln1_b, ln2_w, ln2_b, E, T_OWN, FF, R):
    """Host-side: transpose/retile weights, shard src. Returns per-core in_maps."""
    n_et = E // P
    n_ff = FF // P
    OC_W = min(512, E)
    n_oc = E // OC_W
    import ml_dtypes
    bf16 = ml_dtypes.bfloat16
    c = np.ascontiguousarray
    shared = {
        "wq": c(Wq.reshape(n_et, P, n_et, P).transpose(0, 3, 2, 1)).astype(bf16),
        "wk": c(Wk.reshape(n_et, P, n_et, P).transpose(0, 3, 2, 1)).astype(bf16),
        "wv": c(Wv.reshape(n_oc, OC_W, n_et, P).transpose(0, 2, 3, 1)).astype(bf16),
        "wo": c(Wo.reshape(n_et, P, n_et, P).transpose(0, 3, 2, 1)).astype(bf16),
        "w1": c(W1.reshape(2 * n_ff, P, n_et, P).transpose(0, 3, 2, 1)).astype(bf16),
        "w2": c(W2.reshape(n_et, P, n_ff, P).transpose(0, 3, 2, 1)).astype(bf16),
        "b1d": c(b1.reshape(2 * n_ff, P).T),
        "b2d": c(b2.reshape(n_et, P).T),
        "lnv": c(np.concatenate([v.reshape(n_et, P).T for v in
                                 (ln1_w, ln1_b, ln2_w, ln2_b)], axis=1)),
    }
    in_maps = []
    for core in range(N_CORES):
        b, r = core // R, core % R
        m = dict(shared)
        m["srcT"] = c(src[b, r * T_OWN:(r + 1) * T_OWN, :].T)
        in_maps.append(m)
    return in_maps


_CACHE = {}


def _compiled(cfg_key):
    if cfg_key not in _CACHE:
        E, T_OWN, FF, R = cfg_key
        nc = bacc.Bacc("TRN2", target_bir_lowering=False, debug=False,
                       num_devices=N_CORES)
        build(nc, E, T_OWN, FF, R)
        nc.compile()
        _CACHE[cfg_key] = nc
    return _CACHE[cfg_key]


def run(inputs, cfg, trace=False, tmpdir=None, trace_cores=None):
    E, T_OWN, R = cfg["E"], cfg["T_OWN"], cfg["R"]
    nc = _compiled((E, T_OWN, cfg["FF"], R))
    in_maps = prep_inputs(
        np.asarray(inputs["src"], np.float32),
        np.asarray(inputs["Wq"], np.float32), np.asarray(inputs["Wk"], np.float32),
        np.asarray(inputs["Wv"], np.float32), np.asarray(inputs["Wo"], np.float32),
        np.asarray(inputs["W1"], np.float32), np.asarray(inputs["b1"], np.float32),
        np.asarray(inputs["W2"], np.float32), np.asarray(inputs["b2"], np.float32),
        np.asarray(inputs["ln1_w"], np.float32), np.asarray(inputs["ln1_b"], np.float32),
        np.asarray(inputs["ln2_w"], np.float32), np.asarray(inputs["ln2_b"], np.float32),
        E, T_OWN, cfg["FF"], R)
    res = run_bass_kernel_spmd(nc, in_maps, core_ids=list(range(N_CORES)),
                               trace=trace, tmpdir=tmpdir, trace_cores=trace_cores)
    B, T = 8 // R, R * T_OWN
    out = np.empty((B, T, E), np.float32)
    for core in range(N_CORES):
        b, r = core // R, core % R
        out[b, r * T_OWN:(r + 1) * T_OWN, :] = res.results[core]["outT"].T
    return out, res


def kernel(**inputs) -> np.ndarray:
    out, _ = run(inputs, FULL)
    return out
